# revision 21
# baseline (speedup 1.0000x reference)
"""Trainium2 Bass kernel for nn_MultiHeadAttention_8306466750797.

Reference (per batch b):
  q,k,v = split_heads(x@W{q,k,v} + b)        # [NH=4, T=4096, HD=64]
  q_e,k_e = elu(q), elu(k);  q_n,k_n = L2-normalize along HD (+1e-6)
  scores = (q_n @ k_n^T)/8 ; weights = softmax(scores)
  ctx = weights @ v ; out = merge(ctx)@Wo + bo
  y = layernorm(out + x)*gamma + beta        # eps=1e-12

Since q_n,k_n are unit vectors, |s| <= 1/8 (std ~0.016), so
exp(s) = 1 + s to ~1e-4: softmax(s) == (1+s)/sum(1+s) within harness
tolerance (measured end-to-end rel err 6.5e-5 including bf16, same as the
exact-softmax bf16 kernel). That turns attention into a rank-65 form:

  Gaug[i,m] = sum_k [kn|1]_ki [v|1]_km          # [65,65] per head
  [ctx^T; denom] = Gaug^T @ [qn/8; 1]           # one matmul per q-block
  ctx_n = ctx / denom                           # exact normalization

bv is folded host-side into bo (bo_eff = bo + bv@Wo; exact since softmax
weights sum to 1).

Sharding: 8 cores, zero collectives. Core c -> batch b=c//2, query half
c%2 (2048 queries): x[b] for K/V/G, query slice for Q/residual/output.
"""
import sys

sys.path.insert(0, "/opt/trn_rl_repo")

import numpy as np

B, T, H = 4, 4096, 256
NH, HD = 4, 64
TQ = T // 2          # queries per core
N_CORES = 8
NT = T // 128        # 32 key tiles
NQ = TQ // 128       # 16 query tiles

_CACHE = {}


def _build(affine_trivial=False, xpose="pe"):
    import ml_dtypes
    import concourse.bass as bass
    import concourse.bacc as bacc
    import concourse.mybir as mybir
    import concourse.tile as tile

    F32 = mybir.dt.float32
    BF16 = mybir.dt.bfloat16
    AF = mybir.ActivationFunctionType
    OP = mybir.AluOpType
    bf = ml_dtypes.bfloat16

    nc = bacc.Bacc("TRN2", target_bir_lowering=False, debug=False)

    xkv_d = nc.dram_tensor("xkv", [T, H], F32, kind="ExternalInput")
    xq_d = nc.dram_tensor("xq", [TQ, H], F32, kind="ExternalInput")
    wq_d = nc.dram_tensor("Wq", [H, H], F32, kind="ExternalInput")
    wk_d = nc.dram_tensor("Wk", [H, H], F32, kind="ExternalInput")
    wv_d = nc.dram_tensor("Wv", [H, H], F32, kind="ExternalInput")
    wo_d = nc.dram_tensor("Wo", [H, H], F32, kind="ExternalInput")
    bq_d = nc.dram_tensor("bq", [H], F32, kind="ExternalInput")
    bk_d = nc.dram_tensor("bk", [H], F32, kind="ExternalInput")
    bo_d = nc.dram_tensor("bo", [H], F32, kind="ExternalInput")  # bo + bv@Wo
    ga_d = nc.dram_tensor("gamma", [H], F32, kind="ExternalInput")
    be_d = nc.dram_tensor("beta", [H], F32, kind="ExternalInput")
    out_d = nc.dram_tensor("out", [TQ, H], F32, kind="ExternalOutput")

    ident_np = np.eye(128, dtype=np.float32)
    identb_np = np.eye(128, dtype=bf)
    ones65_np = np.zeros((65, 512), dtype=bf)  # row 64 = ones (p64-aligned operands)
    ones65_np[64, :] = 1
    ones1_np = np.ones((1, 128), dtype=bf)
    onesq_np = np.ones((1, TQ), dtype=bf)

    ident_i = nc.inline_tensor(ident_np, name="c_ident")
    identb_i = nc.inline_tensor(identb_np, name="c_identb")
    ones65_i = nc.inline_tensor(ones65_np, name="c_ones65")
    ones1_i = nc.inline_tensor(ones1_np, name="c_ones1")
    onesq_i = nc.inline_tensor(onesq_np, name="c_onesq")

    def bcast_ap(handle_1d):
        ap = handle_1d[:]
        return bass.AP(tensor=ap.tensor, offset=ap.offset, ap=[[0, 128], *ap.ap])

    with tile.TileContext(nc) as tc:
        with (
            tc.tile_pool(name="const", bufs=1) as const,
            tc.tile_pool(name="wstage", bufs=2) as wstage,
            tc.tile_pool(name="persist", bufs=1) as persist,
            tc.tile_pool(name="sbA", bufs=4) as sbA,
            tc.tile_pool(name="sbB", bufs=4) as sbB,
            tc.tile_pool(name="sbC", bufs=4) as sbC,
            tc.tile_pool(name="sbD", bufs=3) as sbD,
            tc.tile_pool(name="ps_mm", bufs=3, space="PSUM") as ps_mm,
            tc.tile_pool(name="ps_g", bufs=2, space="PSUM") as ps_g,
            tc.tile_pool(name="ps_ctx", bufs=3, space="PSUM") as ps_ctx,
        ):
            # ---------------- constants ----------------
            ident = const.tile([128, 128], F32)
            nc.sync.dma_start(ident[:], ident_i[:])
            identb = const.tile([128, 128], BF16)
            nc.sync.dma_start(identb[:], identb_i[:])
            ones65 = const.tile([65, 512], BF16)
            nc.sync.dma_start(ones65[:], ones65_i[:])
            ones1 = const.tile([1, 128], BF16)
            nc.sync.dma_start(ones1[:], ones1_i[:])

            w_bf = {}
            for name, wd in (("q", wq_d), ("k", wk_d), ("v", wv_d), ("o", wo_d)):
                stg = wstage.tile([128, 2, H], F32, tag="wstg")
                nc.sync.dma_start(stg[:], wd.rearrange("(a p) n -> p a n", p=128))
                wb = const.tile([128, 2, H], BF16, tag=f"w{name}")
                nc.vector.tensor_copy(wb[:], stg[:])
                w_bf[name] = wb

            # bias rows [1, 256] bf16 for PE ones-row bias folding
            def bias_row(name, d_handle):
                stg = wstage.tile([1, H], F32, tag="brow_stg", name=f"stg_{name}")
                nc.sync.dma_start(stg[:], d_handle.rearrange("(a h) -> a h", a=1))
                row = const.tile([1, H], BF16, tag=f"brow_{name}", name=f"brow_{name}")
                nc.vector.tensor_copy(row[:], stg[:])
                return row

            bq_row = bias_row("q", bq_d)
            bk_row = bias_row("k", bk_d)
            bo_row = bias_row("o", bo_d)
            # free-dim broadcast rows [128, 256]
            if not affine_trivial:
                ga_bc = const.tile([128, H], F32, tag="ga_bc")
                nc.gpsimd.dma_start(ga_bc[:], bcast_ap(ga_d))
                be_bc = const.tile([128, H], F32, tag="be_bc")
                nc.gpsimd.dma_start(be_bc[:], bcast_ap(be_d))

            eps12 = const.tile([128, 1], F32, tag="eps12")
            nc.vector.memset(eps12[:], 1e-12)

            # ---------------- persistent tensors ----------------
            xT = [persist.tile([128, T], BF16, tag=f"xT{a}", name=f"xT{a}") for a in range(2)]
            xqT = [persist.tile([128, TQ], BF16, tag=f"xqT{a}", name=f"xqT{a}") for a in range(2)]
            xq_all = persist.tile([128, NQ, H], F32, tag="xq_all")
            ke_k = persist.tile([128, NT, H], BF16, tag="ke_k")     # elu(K), natural
            ke_q = persist.tile([128, NQ, H], BF16, tag="ke_q")     # elu(Q), natural
            rs_k = persist.tile([128, NT, NH], BF16, tag="rs_k")    # per-head sumsq
            rs_q = persist.tile([128, NQ, NH], BF16, tag="rs_q")
            # [Kn|1] and [V|1] per (tile, head) with ones column
            k_all = persist.tile([128, NT, NH, HD + 1], BF16, tag="k_all")
            v_all = persist.tile([128, NT, NH, HD + 1], BF16, tag="v_all")
            nc.gpsimd.memset(k_all[:, :, :, HD : HD + 1], 1.0)
            nc.gpsimd.memset(v_all[:, :, :, HD : HD + 1], 1.0)
            # [Qn/8 ; 1] transposed per head: rows 0-63 qn^T, row 64 ones
            qaug = [persist.tile([65, TQ], BF16, tag=f"qaug{h}", name=f"qaug{h}")
                    for h in range(NH)]
            for h in range(NH):
                nc.sync.dma_start(qaug[h][64:65, :], onesq_i[:])
            ctxT = [persist.tile([128, TQ], BF16, tag=f"ctxT{a}", name=f"ctxT{a}")
                    for a in range(2)]
            g_sb = [persist.tile([65, 65], BF16, tag=f"g{h}", name=f"g{h}")
                    for h in range(NH)]

            # ---------------- stage A: load x, transpose (PE f32), cast on copy ----------------
            def load_transpose(src_slice, dst_xT, tcol):
                if xpose == "dma":
                    xb = sbA.tile([128, H], BF16, tag="xldb")
                    nc.scalar.copy(xb[:], src_slice)
                    for a in range(2):
                        nc.sync.dma_start_transpose(
                            dst_xT[a][:, tcol * 128 : (tcol + 1) * 128],
                            xb[:, a * 128 : (a + 1) * 128],
                        )
                else:
                    for a in range(2):
                        pt = ps_g.tile([128, 128], F32, tag="g")
                        nc.tensor.transpose(pt[:], src_slice[:, a * 128 : (a + 1) * 128], ident[:])
                        nc.vector.tensor_copy(
                            dst_xT[a][:, tcol * 128 : (tcol + 1) * 128], pt[:]
                        )

            for t in range(NQ):
                nc.sync.dma_start(xq_all[:, t, :], xq_d[t * 128 : (t + 1) * 128, :])
                load_transpose(xq_all[:, t, :], xqT, t)
            for t in range(NT):
                xt = sbA.tile([128, H], F32, tag="xld")
                nc.sync.dma_start(xt[:], xkv_d[t * 128 : (t + 1) * 128, :])
                load_transpose(xt[:], xT, t)

            # ---------------- stage B pass 1: proj + ELU + sumsq ----------------
            def proj_elu(xTs, n_tiles, wb, b_row, ke_dst, rs_dst):
                for t in range(n_tiles):
                    ps = ps_mm.tile([128, H], F32, tag="mm")
                    for a_in in range(2):
                        nc.tensor.matmul(
                            ps[:],
                            xTs[a_in][:, t * 128 : (t + 1) * 128],
                            wb[:, a_in, :],
                            start=(a_in == 0),
                            stop=False,
                        )
                    # + bias via ones-row rank-1 matmul
                    nc.tensor.matmul(ps[:], ones1[:], b_row[:], start=False, stop=True)
                    # elu(y) = min(exp(y),1)-1 + max(y,0)
                    e = sbB.tile([128, H], BF16, tag="e")
                    nc.scalar.activation(e[:], ps[:], AF.Exp)
                    r = sbB.tile([128, H], BF16, tag="r")
                    nc.scalar.activation(r[:], ps[:], AF.Relu)
                    nc.vector.tensor_scalar(e[:], e[:], 1.0, -1.0, op0=OP.min, op1=OP.add)
                    nc.vector.tensor_tensor(ke_dst[:, t, :], e[:], r[:], op=OP.add)
                    sq = sbB.tile([128, H], BF16, tag="sq")
                    nc.vector.tensor_mul(sq[:], ke_dst[:, t, :], ke_dst[:, t, :])
                    with nc.allow_low_precision("sumsq of 64 bf16 squares; 0.4% on norm is fine"):
                        nc.vector.reduce_sum(
                            rs_dst[:, t, :],
                            sq[:].rearrange("p (h d) -> p h d", d=HD),
                            axis=mybir.AxisListType.X,
                        )

            proj_elu(xqT, NQ, w_bf["q"], bq_row, ke_q, rs_q)
            proj_elu(xT, NT, w_bf["k"], bk_row, ke_k, rs_k)

            # V projection -> v_all (natural bf16)
            for t in range(NT):
                ps = ps_mm.tile([128, H], F32, tag="mm")
                for a_in in range(2):
                    nc.tensor.matmul(
                        ps[:],
                        xT[a_in][:, t * 128 : (t + 1) * 128],
                        w_bf["v"][:, a_in, :],
                        start=(a_in == 0),
                        stop=(a_in == 1),
                    )
                nc.scalar.copy(
                    v_all[:, t, :, 0:HD],
                    ps[:].rearrange("p (h d) -> p h d", d=HD),
                )

            # ---------------- stage B pass 2: batched rsqrt + normalize ----------------
            # K: rn = 1/(sqrt(ss)+1e-6); Q: rn = 1/(8*sqrt(ss)+8e-6) (folds 1/8)
            def norm_apply(rs_src, n_tiles, scale, eps, ke_src, put, chunks=1):
                cs = n_tiles // chunks
                for c in range(chunks):
                    t0c = c * cs
                    sq8 = sbB.tile([128, cs * NH], F32, tag="sq8", name=f"sq8_{scale}_{c}")
                    nc.scalar.activation(
                        sq8[:],
                        rs_src[:, t0c : t0c + cs, :].rearrange("p a b -> p (a b)"),
                        AF.Sqrt, scale=scale,
                    )
                    nc.vector.tensor_scalar(sq8[:], sq8[:], eps, None, op0=OP.add)
                    rn = sbB.tile([128, cs * NH], F32, tag="rn", name=f"rn_{scale}_{c}")
                    nc.vector.reciprocal(rn[:], sq8[:])
                    rnv = rn[:].rearrange("p (a b) -> p a b", b=NH)
                    for t in range(t0c, t0c + cs):
                        for h in range(NH):
                            put(t, h, rnv[:, t - t0c, h : h + 1],
                                ke_src[:, t, 64 * h : 64 * h + 64])

            # Qn natural staging then PE-transpose into qaug
            qn_nat = persist.tile([128, NQ, H], BF16, tag="qn_nat")
            norm_apply(
                rs_q, NQ, 64.0, 8e-6, ke_q,
                lambda t, h, rcol, qcol: nc.vector.tensor_scalar(
                    qn_nat[:, t, 64 * h : 64 * h + 64], qcol, rcol, None, op0=OP.mult
                ),
            )
            norm_apply(
                rs_k, NT, 1.0, 1e-6, ke_k,
                lambda t, h, rcol, kcol: nc.vector.tensor_scalar(
                    k_all[:, t, h, 0:HD], kcol, rcol, None, op0=OP.mult
                ),
                chunks=2,
            )
            for t in range(NQ):
                for a in range(2):
                    if xpose == "dma":
                        qtmp = sbB.tile([128, 128], BF16, tag="qtmp")
                        nc.sync.dma_start_transpose(
                            qtmp[:], qn_nat[:, t, a * 128 : (a + 1) * 128]
                        )
                        for hh in range(2):
                            nc.sync.dma_start(
                                qaug[2 * a + hh][0:64, t * 128 : (t + 1) * 128],
                                qtmp[64 * hh : 64 * hh + 64, :],
                            )
                    else:
                        pt = ps_g.tile([128, 128], BF16, tag="g")
                        nc.tensor.transpose(
                            pt[:], qn_nat[:, t, a * 128 : (a + 1) * 128], identb[:]
                        )
                        for hh in range(2):
                            nc.vector.tensor_copy(
                                qaug[2 * a + hh][0:64, t * 128 : (t + 1) * 128],
                                pt[64 * hh : 64 * hh + 64, :],
                            )

            # ---------------- stage C: linear attention ----------------
            # Gaug[i,m] = sum_k kaug[k,i] vaug[k,m]  (65x65 per head)
            for h in range(NH):
                g_ps = ps_g.tile([65, 65], F32, tag="g")
                for kb in range(NT):
                    nc.tensor.matmul(
                        g_ps[:],
                        k_all[:, kb, h, :],
                        v_all[:, kb, h, :],
                        start=(kb == 0),
                        stop=(kb == NT - 1),
                    )
                nc.vector.tensor_copy(g_sb[h][:], g_ps[:])

            # [ctx^T; denom] = Gaug^T @ qaug, then normalize by denom
            for qb in range(TQ // 512):
                qsl = slice(qb * 512, (qb + 1) * 512)
                for h in range(NH):
                    a, po = h // 2, 64 * (h % 2)
                    ctx_ps = ps_ctx.tile([65, 512], F32, tag="ctx")
                    nc.tensor.matmul(
                        ctx_ps[:], g_sb[h][:], qaug[h][:, qsl], start=True, stop=True
                    )
                    rcb = sbC.tile([65, 512], BF16, tag="rcb")
                    with nc.allow_low_precision("denom ~4096, bf16 recip = 0.4% ctx scale noise"):
                        nc.vector.reciprocal(rcb[64:65, :], ctx_ps[64:65, :])
                    bc_ps = ps_mm.tile([64, 512], F32, tag="mm")
                    nc.tensor.matmul(
                        bc_ps[:], ones65[64:65, 0:64], rcb[64:65, :], start=True, stop=True
                    )
                    bcs = sbC.tile([64, 512], F32, tag="bcs")
                    nc.scalar.copy(bcs[:], bc_ps[:])
                    nc.vector.tensor_mul(ctxT[a][po : po + 64, qsl], ctx_ps[0:64, :], bcs[:])

            # ---------------- stage D: out-proj + residual + layernorm ----------------
            for qt in range(NQ):
                op_ps = ps_mm.tile([128, H], F32, tag="mm")
                for a in range(2):
                    nc.tensor.matmul(
                        op_ps[:],
                        ctxT[a][:, qt * 128 : (qt + 1) * 128],
                        w_bf["o"][:, a, :],
                        start=(a == 0),
                        stop=False,
                    )
                nc.tensor.matmul(op_ps[:], ones1[:], bo_row[:], start=False, stop=True)
                res = sbD.tile([128, H], F32, tag="res")
                nc.vector.tensor_add(res[:], op_ps[:], xq_all[:, qt, :])
                st = sbD.tile([128, 6], F32, tag="st")
                nc.vector.bn_stats(st[:], res[:])
                mv = sbD.tile([128, 2], F32, tag="mv")
                nc.vector.bn_aggr(mv[:], st[:])
                std = sbD.tile([128, 1], F32, tag="std")
                nc.scalar.activation(std[:], mv[:, 1:2], AF.Sqrt, bias=eps12[:])
                rstd = sbD.tile([128, 1], F32, tag="rstd")
                nc.vector.reciprocal(rstd[:], std[:])
                nrm = sbD.tile([128, H], F32, tag="nrm")
                nc.vector.tensor_scalar(
                    nrm[:], res[:], mv[:, 0:1], rstd[:], op0=OP.subtract, op1=OP.mult
                )
                if affine_trivial:
                    nc.sync.dma_start(out_d[qt * 128 : (qt + 1) * 128, :], nrm[:])
                else:
                    nc.vector.tensor_mul(nrm[:], nrm[:], ga_bc[:])
                    ob = sbD.tile([128, H], F32, tag="ob")
                    nc.vector.tensor_add(ob[:], nrm[:], be_bc[:])
                    nc.sync.dma_start(out_d[qt * 128 : (qt + 1) * 128, :], ob[:])

    nc.finalize()
    return nc


def _get_nc(affine_trivial=False, xpose="pe"):
    key = ("nc", affine_trivial, xpose)
    if key not in _CACHE:
        _CACHE[key] = _build(affine_trivial, xpose)
    return _CACHE[key]


def _in_maps(inputs):
    x = np.ascontiguousarray(np.asarray(inputs["x"], dtype=np.float32))
    f32 = lambda k: np.asarray(inputs[k], dtype=np.float32)
    shared = {k: np.ascontiguousarray(f32(k))
              for k in ("Wq", "Wk", "Wv", "Wo", "bq", "bk", "gamma", "beta")}
    # softmax weights sum to 1 => ctx bias bv contributes bv@Wo to out: fold.
    shared["bo"] = np.ascontiguousarray(f32("bo") + f32("bv") @ f32("Wo"))
    maps = []
    for c in range(N_CORES):
        b, half = c // 2, c % 2
        m = dict(shared)
        m["xkv"] = x[b]
        m["xq"] = np.ascontiguousarray(x[b, half * TQ : (half + 1) * TQ])
        maps.append(m)
    return maps


def kernel(**inputs):
    from concourse.bass_utils import run_bass_kernel_spmd

    trivial = bool(
        np.all(np.asarray(inputs["gamma"]) == 1.0)
        and np.all(np.asarray(inputs["beta"]) == 0.0)
    )
    nc = _get_nc(trivial)
    res = run_bass_kernel_spmd(nc, _in_maps(inputs), core_ids=list(range(N_CORES)))
    y = np.empty((B, T, H), dtype=np.float32)
    for c in range(N_CORES):
        b, half = c // 2, c % 2
        y[b, half * TQ : (half + 1) * TQ] = res.results[c]["out"]
    return y


# revision 31
# speedup vs baseline: 93.8060x; 93.8060x over previous
"""Trainium2 Bass kernel for nn_MultiHeadAttention_8306466750797.

Reference (per batch b):
  q,k,v = split_heads(x@W{q,k,v} + b)        # [NH=4, T=4096, HD=64]
  q_e,k_e = elu(q), elu(k);  q_n,k_n = L2-normalize along HD (+1e-6)
  scores = (q_n @ k_n^T)/8 ; weights = softmax(scores)
  ctx = weights @ v ; out = merge(ctx)@Wo + bo
  y = layernorm(out + x)*gamma + beta        # eps=1e-12

Since q_n,k_n are unit vectors, |s| <= 1/8 (std ~0.016), so
exp(s) = 1 + s to ~1e-4: softmax(s) == (1+s)/sum(1+s) within harness
tolerance (measured end-to-end rel err 6.5e-5 including bf16, same as the
exact-softmax bf16 kernel). That turns attention into a rank-65 form:

  Gaug[i,m] = sum_k [kn|1]_ki [v|1]_km          # [65,65] per head
  [ctx^T; denom] = Gaug^T @ [qn/8; 1]           # one matmul per q-block
  ctx_n = ctx / denom                           # exact normalization

bv is folded host-side into bo (bo_eff = bo + bv@Wo; exact since softmax
weights sum to 1).

Sharding: 8 cores, zero collectives. Core c -> batch b=c//2, query half
c%2 (2048 queries): x[b] for K/V/G, query slice for Q/residual/output.
"""
import sys

sys.path.insert(0, "/opt/trn_rl_repo")

import numpy as np

B, T, H = 4, 4096, 256
NH, HD = 4, 64
TQ = T // 2          # queries per core
N_CORES = 8
NT = T // 128        # 32 key tiles
NQ = TQ // 128       # 16 query tiles

_CACHE = {}


def _build(affine_trivial=False, xpose="pe"):
    import ml_dtypes
    import concourse.bass as bass
    import concourse.bacc as bacc
    import concourse.mybir as mybir
    import concourse.tile as tile

    F32 = mybir.dt.float32
    BF16 = mybir.dt.bfloat16
    AF = mybir.ActivationFunctionType
    OP = mybir.AluOpType
    bf = ml_dtypes.bfloat16

    nc = bacc.Bacc("TRN2", target_bir_lowering=False, debug=False)

    xkv_d = nc.dram_tensor("xkv", [T, H], F32, kind="ExternalInput")
    xq_d = nc.dram_tensor("xq", [TQ, H], F32, kind="ExternalInput")
    wq_d = nc.dram_tensor("Wq", [H, H], F32, kind="ExternalInput")
    wk_d = nc.dram_tensor("Wk", [H, H], F32, kind="ExternalInput")
    wv_d = nc.dram_tensor("Wv", [H, H], F32, kind="ExternalInput")
    wo_d = nc.dram_tensor("Wo", [H, H], F32, kind="ExternalInput")
    bq_d = nc.dram_tensor("bq", [H], F32, kind="ExternalInput")
    bk_d = nc.dram_tensor("bk", [H], F32, kind="ExternalInput")
    bo_d = nc.dram_tensor("bo", [H], F32, kind="ExternalInput")  # bo + bv@Wo
    ga_d = nc.dram_tensor("gamma", [H], F32, kind="ExternalInput")
    be_d = nc.dram_tensor("beta", [H], F32, kind="ExternalInput")
    out_d = nc.dram_tensor("out", [TQ, H], F32, kind="ExternalOutput")

    ident_np = np.eye(128, dtype=np.float32)
    identb_np = np.eye(128, dtype=bf)
    ones65_np = np.zeros((65, 512), dtype=bf)  # row 64 = ones (p64-aligned operands)
    ones65_np[64, :] = 1
    ones1_np = np.ones((1, 128), dtype=bf)
    onesq_np = np.ones((1, TQ), dtype=bf)

    ident_i = nc.inline_tensor(ident_np, name="c_ident")
    identb_i = nc.inline_tensor(identb_np, name="c_identb")
    ones65_i = nc.inline_tensor(ones65_np, name="c_ones65")
    ones1_i = nc.inline_tensor(ones1_np, name="c_ones1")
    onesq_i = nc.inline_tensor(onesq_np, name="c_onesq")

    def bcast_ap(handle_1d):
        ap = handle_1d[:]
        return bass.AP(tensor=ap.tensor, offset=ap.offset, ap=[[0, 128], *ap.ap])

    with tile.TileContext(nc) as tc:
        with (
            tc.tile_pool(name="const", bufs=1) as const,
            tc.tile_pool(name="wstage", bufs=2) as wstage,
            tc.tile_pool(name="persist", bufs=1) as persist,
            tc.tile_pool(name="sbA", bufs=4) as sbA,
            tc.tile_pool(name="sbB", bufs=4) as sbB,
            tc.tile_pool(name="sbC", bufs=4 if affine_trivial else 3) as sbC,
            tc.tile_pool(name="sbD", bufs=3 if affine_trivial else 2) as sbD,
            tc.tile_pool(name="ps_mm", bufs=3, space="PSUM") as ps_mm,
            tc.tile_pool(name="ps_g", bufs=2, space="PSUM") as ps_g,
            tc.tile_pool(name="ps_ctx", bufs=3, space="PSUM") as ps_ctx,
        ):
            # ---------------- constants ----------------
            ident = const.tile([128, 128], F32)
            nc.sync.dma_start(ident[:], ident_i[:])
            identb = const.tile([128, 128], BF16)
            nc.sync.dma_start(identb[:], identb_i[:])
            ones65 = const.tile([65, 512], BF16)
            nc.sync.dma_start(ones65[:], ones65_i[:])
            ones1 = const.tile([1, 128], BF16)
            nc.sync.dma_start(ones1[:], ones1_i[:])

            w_bf = {}
            for name, wd in (("q", wq_d), ("k", wk_d), ("v", wv_d), ("o", wo_d)):
                stg = wstage.tile([128, 2, H], F32, tag="wstg")
                nc.sync.dma_start(stg[:], wd.rearrange("(a p) n -> p a n", p=128))
                wb = const.tile([128, 2, H], BF16, tag=f"w{name}")
                nc.vector.tensor_copy(wb[:], stg[:])
                w_bf[name] = wb

            # bias rows [1, 256] bf16 for PE ones-row bias folding
            def bias_row(name, d_handle):
                stg = wstage.tile([1, H], F32, tag="brow_stg", name=f"stg_{name}")
                nc.sync.dma_start(stg[:], d_handle.rearrange("(a h) -> a h", a=1))
                row = const.tile([1, H], BF16, tag=f"brow_{name}", name=f"brow_{name}")
                nc.vector.tensor_copy(row[:], stg[:])
                return row

            bq_row = bias_row("q", bq_d)
            bk_row = bias_row("k", bk_d)
            bo_row = bias_row("o", bo_d)
            # free-dim broadcast rows [128, 256]
            if not affine_trivial:
                ga_bc = const.tile([128, H], F32, tag="ga_bc")
                nc.gpsimd.dma_start(ga_bc[:], bcast_ap(ga_d))
                be_bc = const.tile([128, H], F32, tag="be_bc")
                nc.gpsimd.dma_start(be_bc[:], bcast_ap(be_d))

            eps12 = const.tile([128, 1], F32, tag="eps12")
            nc.vector.memset(eps12[:], 1e-12)

            # ---------------- persistent tensors ----------------
            xT_all = persist.tile([128, 2, T], BF16, tag="xT_all")
            xqT_all = persist.tile([128, 2, TQ], BF16, tag="xqT_all")
            xT = [xT_all[:, a, :] for a in range(2)]
            xqT = [xqT_all[:, a, :] for a in range(2)]
            xq_all = persist.tile([128, NQ, H], F32, tag="xq_all")
            ke_k = persist.tile([128, NT, H], BF16, tag="ke_k")     # elu(K), natural
            ke_q = persist.tile([128, NQ, H], BF16, tag="ke_q")     # elu(Q), natural
            rs_k = persist.tile([128, NT, NH], BF16, tag="rs_k")    # per-head sumsq
            rs_q = persist.tile([128, NQ, NH], BF16, tag="rs_q")
            # [Kn|1] and [V|1] per (tile, head) with ones column
            k_all = persist.tile([128, NT, NH, HD + 1], BF16, tag="k_all")
            v_all = persist.tile([128, NT, NH, HD + 1], BF16, tag="v_all")
            nc.gpsimd.memset(k_all[:, :, :, HD : HD + 1], 1.0)
            nc.gpsimd.memset(v_all[:, :, :, HD : HD + 1], 1.0)
            # [Qn/8 ; 1] transposed per head: rows 0-63 qn^T, row 64 ones
            qaug = [persist.tile([65, TQ], BF16, tag=f"qaug{h}", name=f"qaug{h}")
                    for h in range(NH)]
            for h in range(NH):
                nc.sync.dma_start(qaug[h][64:65, :], onesq_i[:])
            ctxT = [persist.tile([128, TQ], BF16, tag=f"ctxT{a}", name=f"ctxT{a}")
                    for a in range(2)]
            g_sb = [persist.tile([65, 65], BF16, tag=f"g{h}", name=f"g{h}")
                    for h in range(NH)]

            # ---------------- stage A: load x, transpose (PE f32), cast on copy ----------------
            def load_transpose(src_slice, dst_xT, tcol):
                # dst_xT: [128, 2, T] combined tile
                if xpose == "dma":
                    xb = sbA.tile([128, H], BF16, tag="xldb")
                    nc.scalar.copy(xb[:], src_slice)
                    for a in range(2):
                        nc.sync.dma_start_transpose(
                            dst_xT[:, a, tcol * 128 : (tcol + 1) * 128],
                            xb[:, a * 128 : (a + 1) * 128],
                        )
                else:
                    pt = ps_g.tile([128, 2, 128], F32, tag="g")
                    for a in range(2):
                        nc.tensor.transpose(
                            pt[:, a, :], src_slice[:, a * 128 : (a + 1) * 128], ident[:]
                        )
                    nc.vector.tensor_copy(dst_xT[:, :, tcol * 128 : (tcol + 1) * 128], pt[:])

            for c in range(NQ // 8):
                nc.sync.dma_start(
                    xq_all[:, 8 * c : 8 * (c + 1), :],
                    xq_d[1024 * c : 1024 * (c + 1), :].rearrange("(n p) h -> p n h", p=128),
                )
            for t in range(NQ):
                load_transpose(xq_all[:, t, :], xqT_all, t)
            for c in range(NT // 8):
                x8 = sbA.tile([128, 8, H], F32, tag="xld8")
                nc.sync.dma_start(
                    x8[:],
                    xkv_d[1024 * c : 1024 * (c + 1), :].rearrange("(n p) h -> p n h", p=128),
                )
                for j in range(8):
                    load_transpose(x8[:, j, :], xT_all, 8 * c + j)

            # ---------------- stage B pass 1: proj + ELU + sumsq ----------------
            def proj_elu(xTs, n_tiles, wb, b_row, ke_dst, rs_dst):
                # two token-tiles share one [128, 2, H] psum so ACT/DVE ops
                # process both per instruction
                for tp in range(n_tiles // 2):
                    ps = ps_mm.tile([128, 2, H], F32, tag="mm")
                    for j in range(2):
                        t = 2 * tp + j
                        for a_in in range(2):
                            nc.tensor.matmul(
                                ps[:, j, :],
                                xTs[a_in][:, t * 128 : (t + 1) * 128],
                                wb[:, a_in, :],
                                start=(a_in == 0),
                                stop=False,
                            )
                        nc.tensor.matmul(ps[:, j, :], ones1[:], b_row[:], start=False, stop=True)
                    # elu(y) = min(exp(y),1)-1 + max(y,0)
                    e = sbB.tile([128, 2, H], BF16, tag="e")
                    nc.scalar.activation(e[:], ps[:], AF.Exp)
                    r = sbB.tile([128, 2, H], BF16, tag="r")
                    nc.scalar.activation(r[:], ps[:], AF.Relu)
                    nc.vector.tensor_scalar(e[:], e[:], 1.0, -1.0, op0=OP.min, op1=OP.add)
                    kslc = ke_dst[:, 2 * tp : 2 * tp + 2, :]
                    nc.vector.tensor_tensor(kslc, e[:], r[:], op=OP.add)
                    if tp % 2 == 1:
                        # square+reduce over a 4-tile span of the persistent ke
                        q4 = ke_dst[:, 2 * tp - 2 : 2 * tp + 2, :]
                        sq = sbB.tile([128, 4, H], BF16, tag="sq", bufs=2)
                        nc.vector.tensor_mul(sq[:], q4, q4)
                        with nc.allow_low_precision("sumsq of 64 bf16 squares"):
                            nc.vector.reduce_sum(
                                rs_dst[:, 2 * tp - 2 : 2 * tp + 2, :],
                                sq[:].rearrange("p a (h d) -> p a h d", d=HD),
                                axis=mybir.AxisListType.X,
                            )

            proj_elu(xqT, NQ, w_bf["q"], bq_row, ke_q, rs_q)
            proj_elu(xT, NT, w_bf["k"], bk_row, ke_k, rs_k)

            # V projection -> v_all (natural bf16), two tiles per psum
            for tp in range(NT // 2):
                ps = ps_mm.tile([128, 2, H], F32, tag="mm")
                for j in range(2):
                    t = 2 * tp + j
                    for a_in in range(2):
                        nc.tensor.matmul(
                            ps[:, j, :],
                            xT[a_in][:, t * 128 : (t + 1) * 128],
                            w_bf["v"][:, a_in, :],
                            start=(a_in == 0),
                            stop=(a_in == 1),
                        )
                nc.scalar.copy(
                    v_all[:, 2 * tp : 2 * tp + 2, :, 0:HD],
                    ps[:].rearrange("p a (h d) -> p a h d", d=HD),
                )

            # ---------------- stage B pass 2: batched rsqrt + normalize ----------------
            # K: rn = 1/(sqrt(ss)+1e-6); Q: rn = 1/(8*sqrt(ss)+8e-6) (folds 1/8)
            def norm_apply(rs_src, n_tiles, scale, eps, ke_src, put, chunks=1):
                cs = n_tiles // chunks
                for c in range(chunks):
                    t0c = c * cs
                    sq8 = sbB.tile([128, cs * NH], F32, tag="sq8", name=f"sq8_{scale}_{c}")
                    nc.scalar.activation(
                        sq8[:],
                        rs_src[:, t0c : t0c + cs, :].rearrange("p a b -> p (a b)"),
                        AF.Sqrt, scale=scale,
                    )
                    nc.vector.tensor_scalar(sq8[:], sq8[:], eps, None, op0=OP.add)
                    rn = sbB.tile([128, cs * NH], F32, tag="rn", name=f"rn_{scale}_{c}")
                    nc.vector.reciprocal(rn[:], sq8[:])
                    rnv = rn[:].rearrange("p (a b) -> p a b", b=NH)
                    for t in range(t0c, t0c + cs):
                        for h in range(NH):
                            put(t, h, rnv[:, t - t0c, h : h + 1],
                                ke_src[:, t, 64 * h : 64 * h + 64])

            # Qn natural staging then PE-transpose into qaug
            qn_nat = persist.tile([128, NQ, H], BF16, tag="qn_nat")
            norm_apply(
                rs_q, NQ, 64.0, 8e-6, ke_q,
                lambda t, h, rcol, qcol: nc.vector.tensor_scalar(
                    qn_nat[:, t, 64 * h : 64 * h + 64], qcol, rcol, None, op0=OP.mult
                ),
            )
            norm_apply(
                rs_k, NT, 1.0, 1e-6, ke_k,
                lambda t, h, rcol, kcol: nc.vector.tensor_scalar(
                    k_all[:, t, h, 0:HD], kcol, rcol, None, op0=OP.mult
                ),
                chunks=2,
            )
            for t in range(NQ):
                for a in range(2):
                    if xpose == "dma":
                        qtmp = sbB.tile([128, 128], BF16, tag="qtmp")
                        nc.sync.dma_start_transpose(
                            qtmp[:], qn_nat[:, t, a * 128 : (a + 1) * 128]
                        )
                        for hh in range(2):
                            nc.sync.dma_start(
                                qaug[2 * a + hh][0:64, t * 128 : (t + 1) * 128],
                                qtmp[64 * hh : 64 * hh + 64, :],
                            )
                    else:
                        pt = ps_g.tile([128, 128], BF16, tag="g")
                        nc.tensor.transpose(
                            pt[:], qn_nat[:, t, a * 128 : (a + 1) * 128], identb[:]
                        )
                        for hh in range(2):
                            nc.vector.tensor_copy(
                                qaug[2 * a + hh][0:64, t * 128 : (t + 1) * 128],
                                pt[64 * hh : 64 * hh + 64, :],
                            )

            # ---------------- stage C: linear attention ----------------
            # Gaug[i,m] = sum_k kaug[k,i] vaug[k,m]  (65x65 per head)
            for h in range(NH):
                g_ps = ps_g.tile([65, 65], F32, tag="g")
                for kb in range(NT):
                    nc.tensor.matmul(
                        g_ps[:],
                        k_all[:, kb, h, :],
                        v_all[:, kb, h, :],
                        start=(kb == 0),
                        stop=(kb == NT - 1),
                    )
                nc.vector.tensor_copy(g_sb[h][:], g_ps[:])

            # [ctx^T; denom] = Gaug^T @ qaug, then normalize by denom
            for qb in range(TQ // 512):
                qsl = slice(qb * 512, (qb + 1) * 512)
                for h in range(NH):
                    a, po = h // 2, 64 * (h % 2)
                    ctx_ps = ps_ctx.tile([65, 512], F32, tag="ctx")
                    nc.tensor.matmul(
                        ctx_ps[:], g_sb[h][:], qaug[h][:, qsl], start=True, stop=True
                    )
                    rcb = sbC.tile([65, 512], BF16, tag="rcb")
                    with nc.allow_low_precision("denom ~4096, bf16 recip = 0.4% ctx scale noise"):
                        nc.vector.reciprocal(rcb[64:65, :], ctx_ps[64:65, :])
                    bc_ps = ps_mm.tile([64, 512], F32, tag="mm")
                    nc.tensor.matmul(
                        bc_ps[:], ones65[64:65, 0:64], rcb[64:65, :], start=True, stop=True
                    )
                    bcs = sbC.tile([64, 512], BF16, tag="bcs")
                    nc.scalar.copy(bcs[:], bc_ps[:])
                    cb = sbC.tile([64, 512], BF16, tag="cb")
                    nc.scalar.copy(cb[:], ctx_ps[0:64, :])
                    nc.vector.tensor_mul(ctxT[a][po : po + 64, qsl], cb[:], bcs[:])

            # ---------------- stage D: out-proj + residual + layernorm ----------------
            for qt in range(NQ):
                op_ps = ps_mm.tile([128, H], F32, tag="mm")
                for a in range(2):
                    nc.tensor.matmul(
                        op_ps[:],
                        ctxT[a][:, qt * 128 : (qt + 1) * 128],
                        w_bf["o"][:, a, :],
                        start=(a == 0),
                        stop=False,
                    )
                nc.tensor.matmul(op_ps[:], ones1[:], bo_row[:], start=False, stop=True)
                res = sbD.tile([128, H], F32, tag="res")
                nc.vector.tensor_add(res[:], op_ps[:], xq_all[:, qt, :])
                st = sbD.tile([128, 6], F32, tag="st")
                nc.vector.bn_stats(st[:], res[:])
                mv = sbD.tile([128, 2], F32, tag="mv")
                nc.vector.bn_aggr(mv[:], st[:])
                std = sbD.tile([128, 1], F32, tag="std")
                nc.scalar.activation(std[:], mv[:, 1:2], AF.Sqrt, bias=eps12[:])
                rstd = sbD.tile([128, 1], F32, tag="rstd")
                nc.vector.reciprocal(rstd[:], std[:])
                nb = sbD.tile([128, 1], F32, tag="nb")
                nc.vector.tensor_scalar(
                    nb[:], mv[:, 0:1], rstd[:, 0:1], -1.0, op0=OP.mult, op1=OP.mult
                )
                nrm = sbD.tile([128, H], F32, tag="nrm")
                nc.scalar.activation(
                    nrm[:], res[:], AF.Identity, bias=nb[:], scale=rstd[:, 0:1]
                )
                if affine_trivial:
                    nc.sync.dma_start(out_d[qt * 128 : (qt + 1) * 128, :], nrm[:])
                else:
                    nc.vector.tensor_mul(nrm[:], nrm[:], ga_bc[:])
                    ob = sbD.tile([128, H], F32, tag="ob")
                    nc.vector.tensor_add(ob[:], nrm[:], be_bc[:])
                    nc.sync.dma_start(out_d[qt * 128 : (qt + 1) * 128, :], ob[:])

    nc.finalize()
    return nc


def _get_nc(affine_trivial=False, xpose="pe"):
    key = ("nc", affine_trivial, xpose)
    if key not in _CACHE:
        _CACHE[key] = _build(affine_trivial, xpose)
    return _CACHE[key]


def _in_maps(inputs):
    x = np.ascontiguousarray(np.asarray(inputs["x"], dtype=np.float32))
    f32 = lambda k: np.asarray(inputs[k], dtype=np.float32)
    shared = {k: np.ascontiguousarray(f32(k))
              for k in ("Wq", "Wk", "Wv", "Wo", "bq", "bk", "gamma", "beta")}
    # softmax weights sum to 1 => ctx bias bv contributes bv@Wo to out: fold.
    shared["bo"] = np.ascontiguousarray(f32("bo") + f32("bv") @ f32("Wo"))
    maps = []
    for c in range(N_CORES):
        b, half = c // 2, c % 2
        m = dict(shared)
        m["xkv"] = x[b]
        m["xq"] = np.ascontiguousarray(x[b, half * TQ : (half + 1) * TQ])
        maps.append(m)
    return maps


def kernel(**inputs):
    from concourse.bass_utils import run_bass_kernel_spmd

    trivial = bool(
        np.all(np.asarray(inputs["gamma"]) == 1.0)
        and np.all(np.asarray(inputs["beta"]) == 0.0)
    )
    nc = _get_nc(trivial)
    res = run_bass_kernel_spmd(nc, _in_maps(inputs), core_ids=list(range(N_CORES)))
    y = np.empty((B, T, H), dtype=np.float32)
    for c in range(N_CORES):
        b, half = c // 2, c % 2
        y[b, half * TQ : (half + 1) * TQ] = res.results[c]["out"]
    return y
